# revision 15
# baseline (speedup 1.0000x reference)
"""Trainium2 Bass kernel for CausalSelfAttention (GQA + RoPE), 8-core SPMD.

Sharding: 8 shards = batch (4) x kv-group-pair (2). Each core owns one batch
element and 2 of the 4 GQA groups (8 of 16 q heads, 2 of 4 kv heads):
  - column-parallel qkv projection (w_attn columns for its heads)
  - full attention for its heads
  - row-parallel c_proj (w_proj rows for its head channels) -> partial sums
Host gathers: out[b] = partial[2b] + partial[2b+1] + b_proj; k/v slices concat.

Device layout strategy (per core):
  phase 1: qkv[t, c]   = x @ w_attn_slice.T + b  (lhsT = xT tiles, rhs = W)
  phase 2: rope on q,k in natural [t, d] layout; PE-transpose q,k -> qT,kT
  phase 3: per head: scoresT[t2, t1] = kT_tile.T @ qT (fp32r), causal mask via
           additive mask tile on diagonal blocks, exp on ACT -> expS (bf16),
           column sums via ones-matmul (PE), yT = v.T-accumulated AV matmul,
           normalize with broadcasted reciprocal.
  phase 4: out_partial[t, :] = yT.T @ w_projT_slice (accumulate over heads)
All big matmuls run as float32r (full PE rate with free dim >= 256).
"""

import os
import numpy as np
from contextlib import ExitStack

B, T, C = 4, 1024, 2048
N_HEAD, N_KV, HD = 16, 4, 128
NQ = 8          # q heads per core
NKVC = 2        # kv heads per core
QCOLS = NQ * HD        # 1024
KCOLS = NKVC * HD      # 256
VCOLS = NKVC * HD      # 256
QKV_COLS = QCOLS + KCOLS + VCOLS   # 1536
KT = C // 128          # 16 k-tiles over input channels
MT = T // 128          # 8 tiles over sequence
SCALE = 1.0 / np.sqrt(HD)
NEG = -1.0e30

_CACHED = {}


def _emit(nc, tc, d):
    import concourse.bass as bass
    from concourse import mybir

    f32 = mybir.dt.float32
    f32r = mybir.dt.float32r
    bf16 = mybir.dt.bfloat16
    Exp = mybir.ActivationFunctionType.Exp
    Copy = mybir.ActivationFunctionType.Copy
    add_op = mybir.AluOpType.add

    ctx = ExitStack()
    with ctx:
        # ---------------- constant pool (whole kernel) ----------------
        pc = ctx.enter_context(tc.tile_pool(name="consts", bufs=1))
        ident = pc.tile([128, 128], f32)
        from concourse.masks import make_identity
        make_identity(nc, ident)

        cmask = pc.tile([128, 128], f32)          # 0 if t2<=t1 else NEG
        nc.sync.dma_start(out=cmask, in_=d["cmask"][:, :])

        cos_sb = pc.tile([128, MT, 64], f32)
        sin_sb = pc.tile([128, MT, 64], f32)
        nc.sync.dma_start(out=cos_sb, in_=d["cos"].rearrange("(m p) j -> p m j", p=128))
        nc.sync.dma_start(out=sin_sb, in_=d["sin"].rearrange("(m p) j -> p m j", p=128))

        ones_r = pc.tile([1, 128], f32r)           # bias matmul stationary
        nc.sync.dma_start(out=ones_r, in_=d["onesr"][:, :].bitcast(f32r))
        ones_c = pc.tile([128, 1], bf16)          # column-sum stationary
        nc.vector.memset(ones_c, 1.0)
        bqkv_sb = pc.tile([1, QKV_COLS], f32r)
        nc.sync.dma_start(out=bqkv_sb, in_=d["bqkv"][:, :].bitcast(f32r))

        # persistent activations: lifetimes overlap but don't nest, so the
        # pools are allocated/released manually at phase boundaries.
        p_qkv = tc.alloc_tile_pool(name="p_qkv", bufs=1)   # phases 1-2
        qkv_sb = p_qkv.tile([128, MT, QKV_COLS], f32)

        # ---------------- phase 1: qkv projection ----------------
        with tc.tile_pool(name="p1", bufs=1) as p1, \
             tc.tile_pool(name="p1w", bufs=2) as p1w, \
             tc.tile_pool(name="ps1", bufs=3, space="PSUM") as ps1:
            xT_sb = p1.tile([128, KT, T], f32r)
            for kt in range(KT):
                nc.sync.dma_start(
                    out=xT_sb[:, kt, :],
                    in_=d["xT"][kt * 128:(kt + 1) * 128, :].bitcast(f32r))

            n_ch = QKV_COLS // 512   # 3 chunks of 512
            for ch in range(n_ch):
                wch = p1w.tile([128, KT, 512], f32r, tag="w")
                for kt in range(KT):
                    nc.sync.dma_start(
                        out=wch[:, kt, :],
                        in_=d["wT"][kt * 128:(kt + 1) * 128,
                                    ch * 512:(ch + 1) * 512].bitcast(f32r))
                for mt in range(MT):
                    ps = ps1.tile([128, 512], f32)
                    for kt in range(KT):
                        nc.tensor.matmul(
                            ps,
                            lhsT=xT_sb[:, kt, mt * 128:(mt + 1) * 128],
                            rhs=wch[:, kt, :],
                            start=(kt == 0), stop=False)
                    nc.tensor.matmul(
                        ps,
                        lhsT=ones_r,
                        rhs=bqkv_sb[:, ch * 512:(ch + 1) * 512],
                        start=False, stop=True)
                    nc.scalar.activation(
                        qkv_sb[:, mt, ch * 512:(ch + 1) * 512], ps, Copy)

        # ---------------- phase 2: rope + transposes + k/v out ----------------
        # rope in natural layout: q cols [0, 1024), k cols [1024, 1280)
        p_qk = tc.alloc_tile_pool(name="p_qk", bufs=1, side="right")  # phases 2-3
        qkT = p_qk.tile([128, NQ + NKVC, T], f32r)   # transposed roped q then k
        v_bf = p_qk.tile([128, MT, VCOLS], bf16)
        with tc.tile_pool(name="p2", bufs=4) as p2, \
             tc.tile_pool(name="ps2", bufs=4, space="PSUM") as ps2:
            for mt in range(MT):
                for (c0, nh) in ((0, NQ), (QCOLS, NKVC)):
                    xv = qkv_sb[:, mt, c0:c0 + nh * HD].rearrange(
                        "p (h j two) -> p h j two", h=nh, two=2)
                    re, im = xv[:, :, :, 0], xv[:, :, :, 1]
                    cos_b = cos_sb[:, mt, :].unsqueeze(1).to_broadcast([128, nh, 64])
                    sin_b = sin_sb[:, mt, :].unsqueeze(1).to_broadcast([128, nh, 64])
                    t_rc = p2.tile([128, nh, 64], f32, tag="t_rc")
                    t_is = p2.tile([128, nh, 64], f32, tag="t_is")
                    t_rs = p2.tile([128, nh, 64], f32, tag="t_rs")
                    t_ic = p2.tile([128, nh, 64], f32, tag="t_ic")
                    nc.vector.tensor_mul(t_rc, re, cos_b)
                    nc.vector.tensor_mul(t_is, im, sin_b)
                    nc.vector.tensor_mul(t_rs, re, sin_b)
                    nc.vector.tensor_mul(t_ic, im, cos_b)
                    nc.vector.tensor_sub(re, t_rc, t_is)
                    nc.vector.tensor_add(im, t_rs, t_ic)

            # post-rope k out, raw v out; bf16 copy of v for AV stationary
            for h in range(NKVC):
                nc.sync.dma_start(
                    out=d["k_out"][h].rearrange("(m p) e -> p m e", p=128),
                    in_=qkv_sb[:, :, QCOLS + h * HD:QCOLS + (h + 1) * HD])
                nc.sync.dma_start(
                    out=d["v_out"][h].rearrange("(m p) e -> p m e", p=128),
                    in_=qkv_sb[:, :, QCOLS + KCOLS + h * HD:QCOLS + KCOLS + (h + 1) * HD])
            for mt in range(MT):
                nc.any.tensor_copy(
                    v_bf[:, mt, :],
                    qkv_sb[:, mt, QCOLS + KCOLS:QCOLS + KCOLS + VCOLS])

            # PE transposes: q heads 0..7 -> qkT[:, h], k heads -> qkT[:, 8+h]
            for hh in range(NQ + NKVC):
                c0 = hh * HD if hh < NQ else QCOLS + (hh - NQ) * HD
                for mt in range(MT):
                    tp = ps2.tile([128, 128], f32)
                    nc.tensor.transpose(tp, qkv_sb[:, mt, c0:c0 + HD], ident)
                    nc.any.tensor_copy(qkT[:, hh, mt * 128:(mt + 1) * 128], tp)

        # ---------------- phase 3: attention ----------------
        p_qkv.release()
        p_y = tc.alloc_tile_pool(name="p_y", bufs=1)       # phases 3-4
        yT_sb = p_y.tile([128, NQ, T], f32r)
        with tc.tile_pool(name="p3", bufs=2) as p3, \
             tc.tile_pool(name="p3r", bufs=2) as p3r, \
             tc.tile_pool(name="ps3s", bufs=2, space="PSUM") as ps3s, \
             tc.tile_pool(name="ps3y", bufs=2, space="PSUM") as ps3y, \
             tc.tile_pool(name="ps3m", bufs=1, space="PSUM") as ps3m:
            for h in range(NQ):
                g = h // 4                      # local kv group
                kTh = qkT[:, NQ + g, :]
                qTh = qkT[:, h, :]
                expS = p3.tile([128, MT, T], bf16, tag="expS")
                # scoresT per (t2 tile, t1 chunk of 256)
                for i2 in range(MT):
                    for q4 in range(i2 // 2, 4):
                        ps = ps3s.tile([128, 256], f32, tag="s")
                        nc.tensor.matmul(
                            ps,
                            lhsT=kTh[:, i2 * 128:(i2 + 1) * 128],
                            rhs=qTh[:, q4 * 256:(q4 + 1) * 256],
                            start=True, stop=True)
                        if q4 == i2 // 2:
                            off = (i2 % 2) * 128
                            nc.vector.tensor_add(
                                ps[:, off:off + 128], ps[:, off:off + 128], cmask)
                        nc.scalar.activation(
                            expS[:, i2, q4 * 256:(q4 + 1) * 256], ps, Exp,
                            scale=float(SCALE))
                    if i2 % 2 == 1:
                        # zero the garbage (t2 tile i2, t1 tile i2-1) block
                        nc.vector.memset(
                            expS[:, i2, (i2 - 1) * 128:i2 * 128], 0.0)

                # column sums via ones-matmul, then yT = AV
                sums = ps3m.tile([1, T], f32, tag="sums")
                yT_ps = ps3y.tile([128, T], f32, tag="yT")
                for q4 in range(4):
                    hi = 2 * q4 + 2            # t2 tiles 0..2q4+1 are valid
                    for i2 in range(hi):
                        nc.tensor.matmul(
                            sums[:, q4 * 256:(q4 + 1) * 256],
                            lhsT=ones_c,
                            rhs=expS[:, i2, q4 * 256:(q4 + 1) * 256],
                            start=(i2 == 0), stop=(i2 == hi - 1))
                    for i2 in range(hi):
                        nc.tensor.matmul(
                            yT_ps[:, q4 * 256:(q4 + 1) * 256],
                            lhsT=v_bf[:, i2, g * HD:(g + 1) * HD],
                            rhs=expS[:, i2, q4 * 256:(q4 + 1) * 256],
                            start=(i2 == 0), stop=(i2 == hi - 1))

                recip = p3r.tile([1, T], f32, tag="recip")
                rbc = p3r.tile([128, T], f32, tag="rbc")
                nc.vector.reciprocal(recip, sums)
                nc.gpsimd.partition_broadcast(rbc, recip, 128)
                nc.vector.tensor_mul(yT_sb[:, h, :], yT_ps, rbc)

        # ---------------- phase 4: output projection (partial) ----------------
        p_qk.release()
        with tc.tile_pool(name="p4w", bufs=1) as p4w, \
             tc.tile_pool(name="p4o", bufs=2) as p4o, \
             tc.tile_pool(name="ps4", bufs=8, space="PSUM") as ps4:
            wp_sb = p4w.tile([128, NQ, C], f32r)
            for h in range(NQ):
                nc.sync.dma_start(out=wp_sb[:, h, :],
                                  in_=d["wp"][h * 128:(h + 1) * 128, :].bitcast(f32r))
            for mt in range(MT):
                out_sb = p4o.tile([128, C], f32, tag="out")
                pos = [ps4.tile([128, 512], f32, tag="po", name=f"po{mt}_{i}")
                       for i in range(4)]
                for h in range(NQ):
                    for ch in range(4):
                        nc.tensor.matmul(
                            pos[ch],
                            lhsT=yT_sb[:, h, mt * 128:(mt + 1) * 128],
                            rhs=wp_sb[:, h, ch * 512:(ch + 1) * 512],
                            start=(h == 0), stop=(h == NQ - 1))
                for ch in range(4):
                    nc.scalar.activation(out_sb[:, ch * 512:(ch + 1) * 512],
                                         pos[ch], Copy)
                nc.sync.dma_start(
                    out=d["out_part"][mt * 128:(mt + 1) * 128, :], in_=out_sb)
        p_y.release()


def build_nc():
    import concourse.tile as tile
    from concourse import bacc, mybir

    f32 = mybir.dt.float32
    nc = bacc.Bacc("TRN2", target_bir_lowering=False, debug=False, num_devices=8)
    d = {
        "xT": nc.dram_tensor("xT", [C, T], f32, kind="ExternalInput").ap(),
        "wT": nc.dram_tensor("wT", [C, QKV_COLS], f32, kind="ExternalInput").ap(),
        "bqkv": nc.dram_tensor("bqkv", [1, QKV_COLS], f32, kind="ExternalInput").ap(),
        "cos": nc.dram_tensor("cos", [T, 64], f32, kind="ExternalInput").ap(),
        "sin": nc.dram_tensor("sin", [T, 64], f32, kind="ExternalInput").ap(),
        "wp": nc.dram_tensor("wp", [QCOLS, C], f32, kind="ExternalInput").ap(),
        "cmask": nc.dram_tensor("cmask", [128, 128], f32, kind="ExternalInput").ap(),
        "onesr": nc.dram_tensor("onesr", [1, 128], f32, kind="ExternalInput").ap(),
        "out_part": nc.dram_tensor("out_part", [T, C], f32, kind="ExternalOutput").ap(),
        "k_out": nc.dram_tensor("k_out", [NKVC, T, HD], f32, kind="ExternalOutput").ap(),
        "v_out": nc.dram_tensor("v_out", [NKVC, T, HD], f32, kind="ExternalOutput").ap(),
    }
    with tile.TileContext(nc) as tc:
        _emit(nc, tc, d)
    nc.compile()
    return nc


def make_in_maps(x, freqs_cos, freqs_sin, w_attn, b_attn, w_proj):
    x = np.asarray(x, np.float32)
    w_attn = np.asarray(w_attn, np.float32)
    b_attn = np.asarray(b_attn, np.float32)
    w_proj = np.asarray(w_proj, np.float32)
    cos = np.ascontiguousarray(np.asarray(freqs_cos, np.float32))
    sin = np.ascontiguousarray(np.asarray(freqs_sin, np.float32))
    cmask = np.where(
        np.arange(128)[:, None] <= np.arange(128)[None, :], 0.0, NEG
    ).astype(np.float32)

    in_maps = []
    for c in range(8):
        b, half = c // 2, c % 2
        qrows = np.arange(half * QCOLS, half * QCOLS + QCOLS)
        krows = np.arange(C + half * KCOLS, C + half * KCOLS + KCOLS)
        vrows = np.arange(C + 2 * KCOLS + half * VCOLS,
                          C + 2 * KCOLS + half * VCOLS + VCOLS)
        rows = np.concatenate([qrows, krows, vrows])
        in_maps.append({
            "xT": np.ascontiguousarray(x[b].T),
            "wT": np.ascontiguousarray(w_attn[rows, :].T),
            "bqkv": np.ascontiguousarray(b_attn[rows][None, :]),
            "cos": cos,
            "sin": sin,
            "wp": np.ascontiguousarray(w_proj[:, half * QCOLS:(half + 1) * QCOLS].T),
            "cmask": cmask,
            "onesr": np.ones((1, 128), np.float32),
        })
    return in_maps


def assemble(results, b_proj):
    b_proj = np.asarray(b_proj, np.float32)
    y = np.empty((B, T, C), np.float32)
    k = np.empty((B, N_KV, T, HD), np.float32)
    v = np.empty((B, N_KV, T, HD), np.float32)
    for b in range(B):
        y[b] = results[2 * b]["out_part"] + results[2 * b + 1]["out_part"] + b_proj
        for half in range(2):
            r = results[2 * b + half]
            k[b, 2 * half:2 * half + 2] = r["k_out"]
            v[b, 2 * half:2 * half + 2] = r["v_out"]
    return y, k, v


def kernel(x, freqs_cos, freqs_sin, w_attn, b_attn, w_proj, b_proj):
    from concourse.bass_utils import run_bass_kernel_spmd

    if "nc" not in _CACHED:
        _CACHED["nc"] = build_nc()
    nc = _CACHED["nc"]
    in_maps = make_in_maps(x, freqs_cos, freqs_sin, w_attn, b_attn, w_proj)
    res = run_bass_kernel_spmd(nc, in_maps, core_ids=list(range(8)))
    return assemble(res.results, b_proj)


# revision 16
# speedup vs baseline: 118.7754x; 118.7754x over previous
"""Trainium2 Bass kernel for CausalSelfAttention (GQA + RoPE), 8-core SPMD.

Sharding: 8 shards = batch (4) x kv-group-pair (2). Each core owns one batch
element and 2 of the 4 GQA groups (8 of 16 q heads, 2 of 4 kv heads):
  - column-parallel qkv projection (w_attn columns for its heads)
  - full attention for its heads
  - row-parallel c_proj (w_proj rows for its head channels) -> partial sums
Host gathers: out[b] = partial[2b] + partial[2b+1] + b_proj; k/v slices concat.

Device layout strategy (per core):
  phase 1: qkv[t, c]   = x @ w_attn_slice.T + b  (lhsT = xT tiles, rhs = W)
  phase 2: rope on q,k in natural [t, d] layout; PE-transpose q,k -> qT,kT
  phase 3: per head: scoresT[t2, t1] = kT_tile.T @ qT (fp32r), causal mask via
           additive mask tile on diagonal blocks, exp on ACT -> expS (bf16),
           column sums via ones-matmul (PE), yT = v.T-accumulated AV matmul,
           normalize with broadcasted reciprocal.
  phase 4: out_partial[t, :] = yT.T @ w_projT_slice (accumulate over heads)
All big matmuls run as float32r (full PE rate with free dim >= 256).
"""

import os
import numpy as np
from contextlib import ExitStack

B, T, C = 4, 1024, 2048
N_HEAD, N_KV, HD = 16, 4, 128
NQ = 8          # q heads per core
NKVC = 2        # kv heads per core
QCOLS = NQ * HD        # 1024
KCOLS = NKVC * HD      # 256
VCOLS = NKVC * HD      # 256
QKV_COLS = QCOLS + KCOLS + VCOLS   # 1536
KT = C // 128          # 16 k-tiles over input channels
MT = T // 128          # 8 tiles over sequence
SCALE = 1.0 / np.sqrt(HD)
NEG = -1.0e30

_CACHED = {}


def _emit(nc, tc, d):
    import concourse.bass as bass
    from concourse import mybir

    f32 = mybir.dt.float32
    f32r = mybir.dt.float32r
    bf16 = mybir.dt.bfloat16
    Exp = mybir.ActivationFunctionType.Exp
    Copy = mybir.ActivationFunctionType.Copy
    add_op = mybir.AluOpType.add

    ctx = ExitStack()
    with ctx:
        # ---------------- constant pool (whole kernel) ----------------
        pc = ctx.enter_context(tc.tile_pool(name="consts", bufs=1))
        ident = pc.tile([128, 128], f32)
        from concourse.masks import make_identity
        make_identity(nc, ident)

        cmask = pc.tile([128, 128], f32)          # 0 if t2<=t1 else NEG
        nc.sync.dma_start(out=cmask, in_=d["cmask"][:, :])

        cos_sb = pc.tile([128, MT, 64], f32)
        sin_sb = pc.tile([128, MT, 64], f32)
        nc.sync.dma_start(out=cos_sb, in_=d["cos"].rearrange("(m p) j -> p m j", p=128))
        nc.sync.dma_start(out=sin_sb, in_=d["sin"].rearrange("(m p) j -> p m j", p=128))

        ones_r = pc.tile([1, 128], f32r)           # bias matmul stationary
        nc.sync.dma_start(out=ones_r, in_=d["onesr"][:, :].bitcast(f32r))
        ones_c = pc.tile([128, 1], bf16)          # column-sum stationary
        nc.vector.memset(ones_c, 1.0)
        bqkv_sb = pc.tile([1, QKV_COLS], f32r)
        nc.sync.dma_start(out=bqkv_sb, in_=d["bqkv"][:, :].bitcast(f32r))

        # persistent activations: lifetimes overlap but don't nest, so the
        # pools are allocated/released manually at phase boundaries.
        p_qkv = tc.alloc_tile_pool(name="p_qkv", bufs=1)   # phases 1-2
        qkv_sb = p_qkv.tile([128, MT, QKV_COLS], f32)

        # ---------------- phase 1: qkv projection ----------------
        with tc.tile_pool(name="p1", bufs=1) as p1, \
             tc.tile_pool(name="p1w", bufs=2) as p1w, \
             tc.tile_pool(name="ps1", bufs=3, space="PSUM") as ps1:
            xT_sb = p1.tile([128, KT, T], f32r)
            for kt in range(KT):
                nc.sync.dma_start(
                    out=xT_sb[:, kt, :],
                    in_=d["xT"][kt * 128:(kt + 1) * 128, :].bitcast(f32r))

            n_ch = QKV_COLS // 512   # 3 chunks of 512
            for ch in range(n_ch):
                wch = p1w.tile([128, KT, 512], f32r, tag="w")
                for kt in range(KT):
                    nc.sync.dma_start(
                        out=wch[:, kt, :],
                        in_=d["wT"][kt * 128:(kt + 1) * 128,
                                    ch * 512:(ch + 1) * 512].bitcast(f32r))
                for mt in range(MT):
                    ps = ps1.tile([128, 512], f32)
                    for kt in range(KT):
                        nc.tensor.matmul(
                            ps,
                            lhsT=xT_sb[:, kt, mt * 128:(mt + 1) * 128],
                            rhs=wch[:, kt, :],
                            start=(kt == 0), stop=False)
                    nc.tensor.matmul(
                        ps,
                        lhsT=ones_r,
                        rhs=bqkv_sb[:, ch * 512:(ch + 1) * 512],
                        start=False, stop=True)
                    nc.scalar.activation(
                        qkv_sb[:, mt, ch * 512:(ch + 1) * 512], ps, Copy)

        # ---------------- phase 2: rope + transposes + k/v out ----------------
        # rope in natural layout: q cols [0, 1024), k cols [1024, 1280)
        p_qk = tc.alloc_tile_pool(name="p_qk", bufs=1, side="right")  # phases 2-3
        qkT = p_qk.tile([128, NQ + NKVC, T], f32r)   # transposed roped q then k
        v_bf = p_qk.tile([128, MT, VCOLS], bf16)
        with tc.tile_pool(name="p2", bufs=4) as p2, \
             tc.tile_pool(name="ps2", bufs=4, space="PSUM") as ps2:
            for mt in range(MT):
                for (c0, nh) in ((0, NQ), (QCOLS, NKVC)):
                    xv = qkv_sb[:, mt, c0:c0 + nh * HD].rearrange(
                        "p (h j two) -> p h j two", h=nh, two=2)
                    re, im = xv[:, :, :, 0], xv[:, :, :, 1]
                    cos_b = cos_sb[:, mt, :].unsqueeze(1).to_broadcast([128, nh, 64])
                    sin_b = sin_sb[:, mt, :].unsqueeze(1).to_broadcast([128, nh, 64])
                    t_rc = p2.tile([128, nh, 64], f32, tag="t_rc")
                    t_is = p2.tile([128, nh, 64], f32, tag="t_is")
                    t_rs = p2.tile([128, nh, 64], f32, tag="t_rs")
                    t_ic = p2.tile([128, nh, 64], f32, tag="t_ic")
                    nc.vector.tensor_mul(t_rc, re, cos_b)
                    nc.vector.tensor_mul(t_is, im, sin_b)
                    nc.vector.tensor_mul(t_rs, re, sin_b)
                    nc.vector.tensor_mul(t_ic, im, cos_b)
                    nc.vector.tensor_sub(re, t_rc, t_is)
                    nc.vector.tensor_add(im, t_rs, t_ic)

            # post-rope k out, raw v out; bf16 copy of v for AV stationary
            for h in range(NKVC):
                nc.sync.dma_start(
                    out=d["k_out"][h].rearrange("(m p) e -> p m e", p=128),
                    in_=qkv_sb[:, :, QCOLS + h * HD:QCOLS + (h + 1) * HD])
                nc.sync.dma_start(
                    out=d["v_out"][h].rearrange("(m p) e -> p m e", p=128),
                    in_=qkv_sb[:, :, QCOLS + KCOLS + h * HD:QCOLS + KCOLS + (h + 1) * HD])
            for mt in range(MT):
                nc.any.tensor_copy(
                    v_bf[:, mt, :],
                    qkv_sb[:, mt, QCOLS + KCOLS:QCOLS + KCOLS + VCOLS])

            # PE transposes: q heads 0..7 -> qkT[:, h], k heads -> qkT[:, 8+h]
            for hh in range(NQ + NKVC):
                c0 = hh * HD if hh < NQ else QCOLS + (hh - NQ) * HD
                for mt in range(MT):
                    tp = ps2.tile([128, 128], f32)
                    nc.tensor.transpose(tp, qkv_sb[:, mt, c0:c0 + HD], ident)
                    nc.any.tensor_copy(qkT[:, hh, mt * 128:(mt + 1) * 128], tp)

        # ---------------- phase 3: attention ----------------
        p_qkv.release()
        p_y = tc.alloc_tile_pool(name="p_y", bufs=1)       # phases 3-4
        yT_sb = p_y.tile([128, NQ, T], f32r)
        with tc.tile_pool(name="p3", bufs=2) as p3, \
             tc.tile_pool(name="p3r", bufs=2) as p3r, \
             tc.tile_pool(name="ps3s", bufs=2, space="PSUM") as ps3s, \
             tc.tile_pool(name="ps3y", bufs=2, space="PSUM") as ps3y, \
             tc.tile_pool(name="ps3m", bufs=1, space="PSUM") as ps3m:
            for h in range(NQ):
                g = h // 4                      # local kv group
                kTh = qkT[:, NQ + g, :]
                qTh = qkT[:, h, :]
                expS = p3.tile([128, MT, T], bf16, tag="expS")
                # scoresT per (t2 tile, t1 chunk of 256)
                for i2 in range(MT):
                    for q4 in range(i2 // 2, 4):
                        ps = ps3s.tile([128, 256], f32, tag="s")
                        nc.tensor.matmul(
                            ps,
                            lhsT=kTh[:, i2 * 128:(i2 + 1) * 128],
                            rhs=qTh[:, q4 * 256:(q4 + 1) * 256],
                            start=True, stop=True)
                        if q4 == i2 // 2:
                            off = (i2 % 2) * 128
                            nc.vector.tensor_add(
                                ps[:, off:off + 128], ps[:, off:off + 128], cmask)
                        nc.scalar.activation(
                            expS[:, i2, q4 * 256:(q4 + 1) * 256], ps, Exp,
                            scale=float(SCALE))
                    if i2 % 2 == 1:
                        # zero the garbage (t2 tile i2, t1 tile i2-1) block
                        nc.vector.memset(
                            expS[:, i2, (i2 - 1) * 128:i2 * 128], 0.0)

                # column sums via ones-matmul, then yT = AV
                sums = ps3m.tile([1, T], f32, tag="sums")
                yT_ps = ps3y.tile([128, T], f32, tag="yT")
                for q4 in range(4):
                    hi = 2 * q4 + 2            # t2 tiles 0..2q4+1 are valid
                    for i2 in range(hi):
                        nc.tensor.matmul(
                            sums[:, q4 * 256:(q4 + 1) * 256],
                            lhsT=ones_c,
                            rhs=expS[:, i2, q4 * 256:(q4 + 1) * 256],
                            start=(i2 == 0), stop=(i2 == hi - 1))
                    for i2 in range(hi):
                        nc.tensor.matmul(
                            yT_ps[:, q4 * 256:(q4 + 1) * 256],
                            lhsT=v_bf[:, i2, g * HD:(g + 1) * HD],
                            rhs=expS[:, i2, q4 * 256:(q4 + 1) * 256],
                            start=(i2 == 0), stop=(i2 == hi - 1))

                recip = p3r.tile([1, T], f32, tag="recip")
                rbc = p3r.tile([128, T], f32, tag="rbc")
                nc.vector.reciprocal(recip, sums)
                nc.gpsimd.partition_broadcast(rbc, recip, 128)
                nc.vector.tensor_mul(yT_sb[:, h, :], yT_ps, rbc)

        # ---------------- phase 4: output projection (partial) ----------------
        p_qk.release()
        with tc.tile_pool(name="p4w", bufs=1) as p4w, \
             tc.tile_pool(name="p4o", bufs=2) as p4o, \
             tc.tile_pool(name="ps4", bufs=8, space="PSUM") as ps4:
            wp_sb = p4w.tile([128, NQ, C], f32r)
            for h in range(NQ):
                nc.sync.dma_start(out=wp_sb[:, h, :],
                                  in_=d["wp"][h * 128:(h + 1) * 128, :].bitcast(f32r))
            for mt in range(MT):
                out_sb = p4o.tile([128, C], f32, tag="out")
                pos = [ps4.tile([128, 512], f32, tag="po", name=f"po{mt}_{i}")
                       for i in range(4)]
                for h in range(NQ):
                    for ch in range(4):
                        nc.tensor.matmul(
                            pos[ch],
                            lhsT=yT_sb[:, h, mt * 128:(mt + 1) * 128],
                            rhs=wp_sb[:, h, ch * 512:(ch + 1) * 512],
                            start=(h == 0), stop=(h == NQ - 1))
                for ch in range(4):
                    nc.scalar.activation(out_sb[:, ch * 512:(ch + 1) * 512],
                                         pos[ch], Copy)
                nc.sync.dma_start(
                    out=d["out_part"][mt * 128:(mt + 1) * 128, :], in_=out_sb)
        p_y.release()


def build_nc(repeat=1):
    import concourse.tile as tile
    from concourse import bacc, mybir

    f32 = mybir.dt.float32
    nc = bacc.Bacc("TRN2", target_bir_lowering=False, debug=False, num_devices=8)
    d = {
        "xT": nc.dram_tensor("xT", [C, T], f32, kind="ExternalInput").ap(),
        "wT": nc.dram_tensor("wT", [C, QKV_COLS], f32, kind="ExternalInput").ap(),
        "bqkv": nc.dram_tensor("bqkv", [1, QKV_COLS], f32, kind="ExternalInput").ap(),
        "cos": nc.dram_tensor("cos", [T, 64], f32, kind="ExternalInput").ap(),
        "sin": nc.dram_tensor("sin", [T, 64], f32, kind="ExternalInput").ap(),
        "wp": nc.dram_tensor("wp", [QCOLS, C], f32, kind="ExternalInput").ap(),
        "cmask": nc.dram_tensor("cmask", [128, 128], f32, kind="ExternalInput").ap(),
        "onesr": nc.dram_tensor("onesr", [1, 128], f32, kind="ExternalInput").ap(),
        "out_part": nc.dram_tensor("out_part", [T, C], f32, kind="ExternalOutput").ap(),
        "k_out": nc.dram_tensor("k_out", [NKVC, T, HD], f32, kind="ExternalOutput").ap(),
        "v_out": nc.dram_tensor("v_out", [NKVC, T, HD], f32, kind="ExternalOutput").ap(),
    }
    with tile.TileContext(nc) as tc:
        for _ in range(repeat):
            _emit(nc, tc, d)
    nc.compile()
    return nc


def make_in_maps(x, freqs_cos, freqs_sin, w_attn, b_attn, w_proj):
    x = np.asarray(x, np.float32)
    w_attn = np.asarray(w_attn, np.float32)
    b_attn = np.asarray(b_attn, np.float32)
    w_proj = np.asarray(w_proj, np.float32)
    cos = np.ascontiguousarray(np.asarray(freqs_cos, np.float32))
    sin = np.ascontiguousarray(np.asarray(freqs_sin, np.float32))
    cmask = np.where(
        np.arange(128)[:, None] <= np.arange(128)[None, :], 0.0, NEG
    ).astype(np.float32)

    in_maps = []
    for c in range(8):
        b, half = c // 2, c % 2
        qrows = np.arange(half * QCOLS, half * QCOLS + QCOLS)
        krows = np.arange(C + half * KCOLS, C + half * KCOLS + KCOLS)
        vrows = np.arange(C + 2 * KCOLS + half * VCOLS,
                          C + 2 * KCOLS + half * VCOLS + VCOLS)
        rows = np.concatenate([qrows, krows, vrows])
        in_maps.append({
            "xT": np.ascontiguousarray(x[b].T),
            "wT": np.ascontiguousarray(w_attn[rows, :].T),
            "bqkv": np.ascontiguousarray(b_attn[rows][None, :]),
            "cos": cos,
            "sin": sin,
            "wp": np.ascontiguousarray(w_proj[:, half * QCOLS:(half + 1) * QCOLS].T),
            "cmask": cmask,
            "onesr": np.ones((1, 128), np.float32),
        })
    return in_maps


def assemble(results, b_proj):
    b_proj = np.asarray(b_proj, np.float32)
    y = np.empty((B, T, C), np.float32)
    k = np.empty((B, N_KV, T, HD), np.float32)
    v = np.empty((B, N_KV, T, HD), np.float32)
    for b in range(B):
        y[b] = results[2 * b]["out_part"] + results[2 * b + 1]["out_part"] + b_proj
        for half in range(2):
            r = results[2 * b + half]
            k[b, 2 * half:2 * half + 2] = r["k_out"]
            v[b, 2 * half:2 * half + 2] = r["v_out"]
    return y, k, v


def kernel(x, freqs_cos, freqs_sin, w_attn, b_attn, w_proj, b_proj):
    from concourse.bass_utils import run_bass_kernel_spmd

    if "nc" not in _CACHED:
        _CACHED["nc"] = build_nc()
    nc = _CACHED["nc"]
    in_maps = make_in_maps(x, freqs_cos, freqs_sin, w_attn, b_attn, w_proj)
    res = run_bass_kernel_spmd(nc, in_maps, core_ids=list(range(8)))
    return assemble(res.results, b_proj)
